# revision 14
# baseline (speedup 1.0000x reference)
"""Trainium2 Bass kernel for the AttentionLSTM problem.

Strategy: tensor-parallel over the 4H gate dimension across 8 NeuronCores.
Each core owns a 128-column slice of h (and the matching 4x128 gate columns
of Wx/Wh).  Per timestep it computes its slice of the pre-activations
(u = x_t @ Wx accumulated in PSUM, then h_{t-1} @ Wh accumulated on top),
applies the LSTM gates, and all-gathers the transposed h-chunks so every
core has the full h^T for the next step's matmuls.  The x@Wx matmuls have
no dependency on the recurrence, so they are emitted LOOKAHEAD steps early
and fill the PE while the AllGather is in flight.

Matmuls run in float32r (full-rate fp32 PE mode, moving dim 512 >= 256).
"""

import numpy as np

from bass_rust import InstructionNameOrderedSet

import concourse.bass as bass
import concourse.bacc as bacc
import concourse.mybir as mybir
from concourse import tile
from concourse.bass_utils import run_bass_kernel_spmd

F32 = mybir.dt.float32
F32R = mybir.dt.float32r
AF = mybir.ActivationFunctionType


def _ensure_ntff_hook_module():
    """bass_utils imports antenv.axon_hooks for NTFF tracing under axon;
    this image's antenv lacks it.  Provide it, backed by the ctypes hook
    from trn_agent_boot when available (else tracing degrades to a no-op)."""
    import sys
    import types

    if "antenv.axon_hooks" in sys.modules:
        return
    try:
        import antenv.axon_hooks  # noqa: F401
        return
    except ImportError:
        pass
    hook = None
    try:
        from trn_agent_boot.trn_boot import _ntff_profile_via_ctypes
        hook = _ntff_profile_via_ctypes("/opt/axon/libaxon_pjrt.so")
    except Exception:
        hook = None
    mod = types.ModuleType("antenv.axon_hooks")
    mod._hook = hook
    mod.get_axon_ntff_profile_hook = lambda: mod._hook
    mod.set_axon_ntff_profile_hook = lambda h: setattr(mod, "_hook", h)
    sys.modules["antenv.axon_hooks"] = mod


_ensure_ntff_hook_module()

N, T, D, H = 64, 256, 1024, 1024
P = 128                 # SBUF partitions / PE contraction tile
NCORES = 8
CH = H // NCORES        # 128 h-columns owned per core
GC = 4 * CH             # 512 gate columns per core (i,f,o,g slices)
KT = D // P             # 8 contraction tiles
SPAN = 16               # timesteps of x loaded per DMA span (1024 tokens)
LOOKAHEAD = 2           # steps the u-matmuls run ahead of the recurrence

_cached = {}


def _build(with_bias: bool, n_steps: int = T):
    nc = bacc.Bacc("TRN2", target_bir_lowering=False, debug=False,
                   num_devices=NCORES, monotonic_sem_count=2)
    rsem = nc.monotonic_semaphore(0).sem()   # remote-arrival counter (+16/step)
    lsem = nc.monotonic_semaphore(1).sem()   # local send-complete counter (+16/step)

    xT = nc.dram_tensor("xT", [D, T * N], F32R, kind="ExternalInput")
    wx = nc.dram_tensor("wx", [D, GC], F32R, kind="ExternalInput")
    wh = nc.dram_tensor("wh", [D, GC], F32R, kind="ExternalInput")
    ach = nc.dram_tensor("ach", [P, N * 100], F32, kind="ExternalInput")
    ident = nc.dram_tensor("ident", [P, P], F32, kind="ExternalInput")
    if with_bias:
        bvec = nc.dram_tensor("bvec", [1, GC], F32R, kind="ExternalInput")
        ones = nc.dram_tensor("ones", [1, N], F32R, kind="ExternalInput")
    out = nc.dram_tensor("out", [n_steps, N, CH], F32, kind="ExternalOutput")

    rg = [list(range(NCORES))]

    with tile.TileContext(nc) as tc:
        with (
            tc.tile_pool(name="const", bufs=1) as cpool,
            tc.tile_pool(name="x", bufs=2) as xpool,
            tc.tile_pool(name="work", bufs=2) as wpool,
            tc.tile_pool(name="hbuf", bufs=3) as hpool,
            tc.tile_pool(name="ps", bufs=4, space="PSUM") as pspool,
            tc.tile_pool(name="pst", bufs=2, space="PSUM") as pstpool,
            tc.tile_pool(name="dram", bufs=3, space="DRAM") as dpool,
        ):
            # monotonic sems persist across NEFF executions - clear them
            # before any cross-core traffic; the h0 AllGather below acts as
            # the barrier that keeps peers from racing ahead of the clears.
            with tc.tile_critical():
                nc.gpsimd.sem_clear(rsem)
                nc.gpsimd.sem_clear(lsem)
            pid = nc.gpsimd.partition_id()

            # ---- weights / constants ----
            wx_s = cpool.tile([P, KT, GC], F32R)
            wh_s = cpool.tile([P, KT, GC], F32R)
            for kt in range(KT):
                nc.sync.dma_start(out=wx_s[:, kt, :], in_=wx[kt * P:(kt + 1) * P, :])
                nc.sync.dma_start(out=wh_s[:, kt, :], in_=wh[kt * P:(kt + 1) * P, :])
            id_s = cpool.tile([P, P], F32)
            nc.sync.dma_start(out=id_s[:], in_=ident[:])
            if with_bias:
                b_s = cpool.tile([1, GC], F32R)
                ones_s = cpool.tile([1, N], F32R)
                nc.sync.dma_start(out=b_s[:], in_=bvec[:])
                nc.sync.dma_start(out=ones_s[:], in_=ones[:])

            # ---- h0 = mean(A) for this core's 128 h-columns ----
            a_s = cpool.tile([P, N * 100], F32)
            for q in range(4):
                nc.sync.dma_start(out=a_s[:, q * 1600:(q + 1) * 1600],
                                  in_=ach[:, q * 1600:(q + 1) * 1600])
            h0t = cpool.tile([P, N], F32)
            nc.vector.reduce_sum(h0t[:], a_s[:].rearrange("p (n q) -> p n q", q=100),
                                 axis=mybir.AxisListType.X)
            nc.scalar.activation(h0t[:], h0t[:], AF.Copy, bias=0.0, scale=0.01)

            # c0 = h0-chunk in (batch, col) layout
            ps_c0 = pstpool.tile([N, CH], F32, name="ps_hT", tag="ps_hT")
            nc.tensor.transpose(ps_c0[:], h0t[:], id_s[:])
            c_prev = wpool.tile([N, CH], F32, name="c", tag="c")
            nc.vector.tensor_copy(c_prev[:], ps_c0[:])

            # step -1 "exchange": all-gather h0^T so every core has full h0
            h0t_r = cpool.tile([P, N], F32R)
            nc.vector.tensor_copy(h0t_r[:], h0t[:])
            b_in0 = dpool.tile([P, N], F32R, name="b_in", tag="b_in")
            nc.sync.dma_start(out=b_in0[:], in_=h0t_r[:])
            b_out0 = dpool.tile([H, N], F32R, name="b_out", tag="b_out",
                                addr_space="Shared")
            nc.gpsimd.collective_compute(
                "AllGather", mybir.AluOpType.bypass, replica_groups=rg,
                ins=[b_in0[:]], outs=[b_out0[:]])
            hT_prev = hpool.tile([P, KT, N], F32R, name="hT", tag="hT")
            nc.sync.dma_start(out=hT_prev[:],
                              in_=b_out0[:].rearrange("(kt p) n -> p kt n", p=P))

            # ---- main loop (software-pipelined emission) ----
            ps_tiles = {}
            xspan_s = None

            def emit_u(t):
                nonlocal xspan_s
                if t % SPAN == 0:
                    s = t // SPAN
                    xspan_s = xpool.tile([P, KT, SPAN * N], F32R,
                                         name="xspan", tag="xspan")
                    for kt in range(KT):
                        nc.sync.dma_start(
                            out=xspan_s[:, kt, :],
                            in_=xT[kt * P:(kt + 1) * P,
                                   s * SPAN * N:(s + 1) * SPAN * N])
                ps = pspool.tile([N, GC], F32, name="ps_a", tag="ps_a")
                ps_tiles[t] = (ps, xspan_s)
                j = t % SPAN
                for kt in range(KT):
                    umm = nc.tensor.matmul(
                        ps[:], lhsT=xspan_s[:, kt, j * N:(j + 1) * N],
                        rhs=wx_s[:, kt, :],
                        start=(kt == 0), stop=False, skip_group_check=True)
                    ustate["last_mm"] = umm
                if with_bias:
                    nc.tensor.matmul(ps[:], lhsT=ones_s[:], rhs=b_s[:],
                                     start=False, stop=False,
                                     skip_group_check=True)

            bstate = {"n": 0}
            ustate = {"last_mm": None}
            prev_dve = None
            post_waits = []

            def emit_step(t):
                nonlocal c_prev, hT_prev, prev_dve
                ps, _ = ps_tiles.pop(t)
                for kt in range(KT):
                    mm = nc.tensor.matmul(
                        ps[:], lhsT=hT_prev[:, kt, :], rhs=wh_s[:, kt, :],
                        start=False, stop=(kt == KT - 1), skip_group_check=True)
                # gates: columns [i(128) f(128) o(128) g(128)]
                sig = wpool.tile([N, 3 * CH], F32, name="sig", tag="sig")
                nc.scalar.activation(sig[:], ps[:, 0:3 * CH], AF.Sigmoid)
                gg = wpool.tile([N, CH], F32, name="gg", tag="gg")
                nc.scalar.activation(gg[:], ps[:, 3 * CH:4 * CH], AF.Tanh)
                ig = wpool.tile([N, CH], F32, name="ig", tag="ig")
                nc.vector.tensor_mul(out=ig[:], in0=sig[:, 0:CH], in1=gg[:])
                c_new = wpool.tile([N, CH], F32, name="c", tag="c")
                nc.vector.tensor_mul(out=c_new[:], in0=sig[:, CH:2 * CH], in1=c_prev[:])
                nc.vector.tensor_add(out=c_new[:], in0=c_new[:], in1=ig[:])
                tch = wpool.tile([N, CH], F32, name="tch", tag="tch")
                nc.scalar.activation(tch[:], c_new[:], AF.Tanh)
                h_new = hpool.tile([N, CH], F32, name="h", tag="h")
                hmul = nc.vector.tensor_mul(out=h_new[:], in0=sig[:, 2 * CH:3 * CH], in1=tch[:])
                prev_dve = hmul
                nc.sync.dma_start(out=out[t], in_=h_new[:])
                c_prev = c_new
                if t == n_steps - 1:
                    return
                # exchange h^T: every core broadcasts its transposed chunk
                # into slot `pid` of every peer's hT tile (SBUF->SBUF).
                ps_hT = pstpool.tile([P, N], F32, name="ps_hT", tag="ps_hT")
                nc.tensor.transpose(ps_hT[:], h_new[:], id_s[0:N, 0:N])
                hT_sb = wpool.tile([P, N], F32R, name="hT_sb", tag="hT_sb")
                nc.vector.tensor_copy(hT_sb[:], ps_hT[:])
                b_in = dpool.tile([P, N], F32R, name="b_in", tag="b_in")
                nc.sync.dma_start(out=b_in[:], in_=hT_sb[:])
                b_out = dpool.tile([H, N], F32R, name="b_out", tag="b_out",
                                   addr_space="Shared")
                nc.gpsimd.collective_compute(
                    "AllGather", mybir.AluOpType.bypass, replica_groups=rg,
                    ins=[b_in[:]], outs=[b_out[:]])
                hT_new = hpool.tile([P, KT, N], F32R, name="hT", tag="hT")
                for kt in range(KT):
                    nc.sync.dma_start(out=hT_new[:, kt, :],
                                      in_=b_out[kt * P:(kt + 1) * P, :])
                hT_prev = hT_new

            for t in range(n_steps + LOOKAHEAD):
                if t - LOOKAHEAD >= 0:
                    emit_step(t - LOOKAHEAD)
                if t < n_steps:
                    emit_u(t)

    for inst, val in post_waits:
        inst.ins.sync_info.on_wait[0].wait_value = val
    nc.compile()
    return nc


def kernel(x, A, Wx, Wh, b):
    x = np.ascontiguousarray(np.asarray(x, dtype=np.float32))
    A = np.ascontiguousarray(np.asarray(A, dtype=np.float32))
    Wx = np.asarray(Wx, dtype=np.float32)
    Wh = np.asarray(Wh, dtype=np.float32)
    b = np.asarray(b, dtype=np.float32)

    import os
    with_bias = bool(np.any(b))
    n_steps = int(os.environ.get("KERNEL_STEPS", T))
    key = (with_bias, n_steps)
    if key not in _cached:
        _cached[key] = _build(with_bias, n_steps)
    nc = _cached[key]

    xT_np = np.ascontiguousarray(x.transpose(2, 1, 0).reshape(D, T * N))
    ident_np = np.eye(P, dtype=np.float32)

    in_maps = []
    for k in range(NCORES):
        cols = np.concatenate([np.arange(g * H + k * CH, g * H + k * CH + CH)
                               for g in range(4)])
        m = {
            "xT": xT_np,
            "wx": np.ascontiguousarray(Wx[:, cols]),
            "wh": np.ascontiguousarray(Wh[:, cols]),
            "ach": np.ascontiguousarray(
                A[:, k * CH:(k + 1) * CH].transpose(1, 0, 2, 3).reshape(P, N * 100)),
            "ident": ident_np,
        }
        if with_bias:
            m["bvec"] = np.ascontiguousarray(b[cols].reshape(1, GC))
            m["ones"] = np.ones((1, N), dtype=np.float32)
        in_maps.append(m)

    res = run_bass_kernel_spmd(nc, in_maps, core_ids=list(range(NCORES)))
    global last_result
    last_result = res

    final = np.empty((N, n_steps, H), dtype=np.float32)
    for k in range(NCORES):
        final[:, :, k * CH:(k + 1) * CH] = res.results[k]["out"].transpose(1, 0, 2)
    return final
